# revision 8
# baseline (speedup 1.0000x reference)
"""BLOOM attention block (QKV proj + alibi causal attention + dense + residual)
as a head-sharded (tensor-parallel) Bass kernel on 8 Trainium2 NeuronCores.

Sharding: 2 heads per core. Per core:
  - qkvT[o, s] = Wsel @ hs.T computed from host-pretransposed hs.T (16 MB,
    replicated) and the core's 768-row slice of W_qkv (transposed, Q rows
    pre-scaled by 1/sqrt(hd)).
  - Attention in transposed orientation: S.T[k, q] = K @ Q.T so the exp output
    is already P.T, which feeds the PV matmul with V (PE-transposed per head)
    as the stationary operand. Softmax uses a fixed per-q shift
    c[q] = max_{allowed k} alibi[k] (exact by shift invariance; no max pass):
    P.T = exp(S.T + alibi[k] - c[q]), l[q] = ones @ P.T (replicated rows),
    ctx.T[d, q] = (V.T)·P.T / l.
  - ctx.T [256, 2048] per core is AllGathered (2 MB/rank) into the full
    [2048, 2048] ctx.T; each core then computes a 256-column shard of the
    dense output as out.T[col, s] = WdT.T @ ctx.T + b + residual.T.
Host assembles the 8 column shards.
"""

import sys

sys.path.insert(0, "/opt/trn_rl_repo")

import math

import numpy as np

import concourse.bass as bass
import concourse.mybir as mybir
import concourse.tile as tile
from concourse.bass_utils import run_bass_kernel_spmd

F32 = mybir.dt.float32
AF = mybir.ActivationFunctionType
ALU = mybir.AluOpType

B, S, H, NH = 1, 2048, 2048, 16
HD = H // NH  # 128
N_CORES = 8
NH_LOC = NH // N_CORES  # 2 heads per core
OPC = 3 * NH_LOC  # 6 qkv output row-blocks of 128 per core
P = 128
QCH = 512  # q chunk (free dim) for attention blocks
NQC = S // QCH  # 4
NKC = S // P  # 16
NSC = S // QCH  # 4  s-chunks in qkv projection
NHC = H // P  # 16  contraction chunks
DCOL = H // N_CORES  # 256 dense output columns per core
NEG_BIG = -1.0e38

_ctr = [0]


def _split_waits(nc, default_limit=1, drain_limit=1):
    """This container's walrus accepts few sync-wait commands per instruction
    (1 for CTRL/Drain, ~2 elsewhere), while Tile attaches one wait per
    upstream proc. Hoist the excess waits onto standalone EventSemaphore
    instructions just before the over-subscribed instruction on the same
    engine (same sequencer => identical blocking semantics)."""
    for bb in nc.main_func.blocks:
        new = []
        changed = False
        for ins in bb.instructions:
            si = ins.sync_info
            ow = list(si.on_wait) if si is not None else []
            lim = drain_limit if isinstance(ins, mybir.InstDrain) else default_limit
            if len(ow) > lim:
                for w in ow[:-lim]:
                    _ctr[0] += 1
                    nop = mybir.InstEventSemaphore(
                        name=f"I-waitsplit-{_ctr[0]}",
                        engine=ins.engine,
                        ins=[],
                        outs=[],
                        sync_info=mybir.SyncInfo(on_wait=[w], on_update=[]),
                    )
                    nc.register_instruction(nop)
                    new.append(nop)
                    changed = True
                ins.sync_info = mybir.SyncInfo(
                    on_wait=ow[-lim:], on_update=list(si.on_update)
                )
            new.append(ins)
        if changed:
            bb.instructions = new


def build_program(block_lists, n_part):
    """block_lists: per qc, list of (kc, mask_tile_idx_or_None), shared by all
    cores/heads (the mask input is head-independent). n_part: number of
    partial-block mask tiles staged in the maskadd input."""
    nc = bass.Bass()

    hsT = nc.dram_tensor("hsT", [H, S], F32, kind="ExternalInput")
    wqkvT = nc.dram_tensor("wqkvT", [H, OPC * P], F32, kind="ExternalInput")
    bqkv = nc.dram_tensor("bqkv", [P, OPC], F32, kind="ExternalInput")
    alibi_b = nc.dram_tensor("alibi_b", [P, NH_LOC * NKC], F32, kind="ExternalInput")
    negc = nc.dram_tensor("negc", [NH_LOC, S], F32, kind="ExternalInput")
    wdT = nc.dram_tensor("wdT", [H, DCOL], F32, kind="ExternalInput")
    bdense = nc.dram_tensor("bdense", [P, DCOL // P], F32, kind="ExternalInput")
    residT = nc.dram_tensor("residT", [DCOL, S], F32, kind="ExternalInput")
    identity = nc.dram_tensor("identity", [P, P], F32, kind="ExternalInput")
    ones128 = nc.dram_tensor("ones128", [P, P], F32, kind="ExternalInput")
    maskadd = (
        nc.dram_tensor("maskadd", [n_part * P, QCH], F32, kind="ExternalInput")
        if n_part
        else None
    )
    outT = nc.dram_tensor("outT", [DCOL, S], F32, kind="ExternalOutput")

    with tile.TileContext(nc) as tc:
        with (
            tc.tile_pool(name="consts", bufs=1) as consts,
            tc.tile_pool(name="qkvsb", bufs=1) as qkvsb_pool,
            tc.tile_pool(name="dram", bufs=1, space="DRAM") as dram_pool,
        ):
            ident_sb = consts.tile([P, P], F32)
            nc.sync.dma_start(ident_sb[:], identity[:])
            ones_sb = consts.tile([P, P], F32)
            nc.sync.dma_start(ones_sb[:], ones128[:])
            bqkv_sb = consts.tile([P, OPC], F32)
            nc.sync.dma_start(bqkv_sb[:], bqkv[:])
            alibi_sb = consts.tile([P, NH_LOC * NKC], F32)
            nc.sync.dma_start(alibi_sb[:], alibi_b[:])
            bdense_sb = consts.tile([P, DCOL // P], F32)
            nc.sync.dma_start(bdense_sb[:], bdense[:])
            # -c[h, q] broadcast to all partitions (DMA stride-0 read)
            negc_sb = []
            for h in range(NH_LOC):
                t = consts.tile([P, S], F32, name=f"negc_sb{h}")
                nc.sync.dma_start(t[:], negc[h : h + 1, :].to_broadcast((P, S)))
                negc_sb.append(t)

            # qkvT[o, s]: 6 row-blocks [128, 2048]
            qkv_sb = [
                qkvsb_pool.tile([P, S], F32, name=f"qkv_sb{ot}") for ot in range(OPC)
            ]

            # ---- Phase 1: fused QKV projection (contraction over H) ----
            with (
                tc.tile_pool(name="wq", bufs=1) as wq_pool,
                tc.tile_pool(name="hst", bufs=20) as hst_pool,
                tc.tile_pool(name="qkvps", bufs=3, space="PSUM") as qkv_ps,
            ):
                wq_sb = []
                for hc in range(NHC):
                    t = wq_pool.tile([P, OPC * P], F32, name=f"wq_sb{hc}")
                    nc.sync.dma_start(t[:], wqkvT[hc * P : (hc + 1) * P, :])
                    wq_sb.append(t)
                for sc in range(NSC):
                    s0 = sc * QCH
                    hs_t = []
                    for hc in range(NHC):
                        t = hst_pool.tile([P, QCH], F32, name="hs_t")
                        nc.sync.dma_start(
                            t[:], hsT[hc * P : (hc + 1) * P, s0 : s0 + QCH]
                        )
                        hs_t.append(t)
                    for ot in range(OPC):
                        ps = qkv_ps.tile([P, QCH], F32, name="qkv_acc")
                        for hc in range(NHC):
                            nc.tensor.matmul(
                                ps[:],
                                wq_sb[hc][:, ot * P : (ot + 1) * P],
                                hs_t[hc][:],
                                start=(hc == 0),
                                stop=(hc == NHC - 1),
                            )
                        nc.scalar.activation(
                            qkv_sb[ot][:, s0 : s0 + QCH],
                            ps[:],
                            AF.Identity,
                            bias=bqkv_sb[:, ot : ot + 1],
                        )

            # ---- Phase 2: attention per head ----
            ctxT_sb = [
                qkvsb_pool.tile([P, S], F32, name=f"ctxT_sb{h}") for h in range(NH_LOC)
            ]
            with (
                tc.tile_pool(name="masks", bufs=1) as mask_pool,
                tc.tile_pool(name="vnat", bufs=1) as vnat_pool,
                tc.tile_pool(name="pt", bufs=16) as pt_pool,
                tc.tile_pool(name="lrec", bufs=2) as lrec_pool,
                tc.tile_pool(name="vtps", bufs=2, space="PSUM") as vt_ps,
                tc.tile_pool(name="stps", bufs=2, space="PSUM") as st_ps,
                tc.tile_pool(name="ctxps", bufs=2, space="PSUM") as ctx_ps,
                tc.tile_pool(name="lps", bufs=2, space="PSUM") as l_ps,
            ):
                mask_sb = []
                if n_part:
                    for mi in range(n_part):
                        t = mask_pool.tile([P, QCH], F32, name=f"mask_sb{mi}")
                        nc.sync.dma_start(t[:], maskadd[mi * P : (mi + 1) * P, :])
                        mask_sb.append(t)
                for h in range(NH_LOC):
                    QT = qkv_sb[3 * h + 0]
                    KT = qkv_sb[3 * h + 1]
                    VT = qkv_sb[3 * h + 2]
                    # V natural [k, d] via PE transpose of VT column blocks
                    vn = []
                    for kc in range(NKC):
                        vp = vt_ps.tile([P, P], F32, name="vt_p")
                        nc.tensor.transpose(
                            vp[:], VT[:, kc * P : (kc + 1) * P], ident_sb[:]
                        )
                        t = vnat_pool.tile([P, P], F32, name=f"vn{h}_{kc}")
                        nc.vector.tensor_copy(t[:], vp[:])
                        vn.append(t)
                    for qc in range(NQC):
                        q0 = qc * QCH
                        kcs = block_lists[qc]
                        pts = {}
                        for kc, mi in kcs:
                            st = st_ps.tile([P, QCH], F32, name="st")
                            nc.tensor.matmul(
                                st[:],
                                KT[:, kc * P : (kc + 1) * P],
                                QT[:, q0 : q0 + QCH],
                                start=True,
                                stop=True,
                            )
                            nc.vector.tensor_tensor(
                                out=st[:],
                                in0=st[:],
                                in1=negc_sb[h][:, q0 : q0 + QCH],
                                op=ALU.add,
                            )
                            if mi is not None:
                                nc.vector.tensor_tensor(
                                    out=st[:],
                                    in0=st[:],
                                    in1=mask_sb[mi][:],
                                    op=ALU.add,
                                )
                            pt = pt_pool.tile([P, QCH], F32, name="pt")
                            col = h * NKC + kc
                            nc.scalar.activation(
                                pt[:],
                                st[:],
                                AF.Exp,
                                bias=alibi_sb[:, col : col + 1],
                            )
                            pts[kc] = pt
                        cps = ctx_ps.tile([P, QCH], F32, name="cacc")
                        for i, (kc, _mi) in enumerate(kcs):
                            nc.tensor.matmul(
                                cps[:],
                                vn[kc][:],
                                pts[kc][:],
                                start=(i == 0),
                                stop=(i == len(kcs) - 1),
                            )
                        lps = l_ps.tile([P, QCH], F32, name="lacc")
                        for i, (kc, _mi) in enumerate(kcs):
                            nc.tensor.matmul(
                                lps[:],
                                ones_sb[:],
                                pts[kc][:],
                                start=(i == 0),
                                stop=(i == len(kcs) - 1),
                            )
                        rec = lrec_pool.tile([P, QCH], F32, name="rec")
                        nc.vector.reciprocal(rec[:], lps[:])
                        nc.vector.tensor_tensor(
                            out=ctxT_sb[h][:, q0 : q0 + QCH],
                            in0=cps[:],
                            in1=rec[:],
                            op=ALU.mult,
                        )

            # ---- Phase 3: AllGather ctx.T across cores ----
            ag_in = dram_pool.tile([NH_LOC * P, S], F32, name="ag_in")
            for h in range(NH_LOC):
                nc.sync.dma_start(ag_in[h * P : (h + 1) * P, :], ctxT_sb[h][:])
            ag_out, _free_ag = tc.tile(
                [H, S], F32, space="DRAM", addr_space="Shared", name="ag_out"
            )
            nc.gpsimd.collective_compute(
                "AllGather",
                ALU.bypass,
                replica_groups=[list(range(N_CORES))],
                ins=[ag_in.opt()],
                outs=[ag_out.opt()],
            )

            # ---- Phase 4: dense column shard + bias + residual ----
            with (
                tc.tile_pool(name="wd", bufs=1) as wd_pool,
                tc.tile_pool(name="ctxf", bufs=3) as ctxf_pool,
                tc.tile_pool(name="residsb", bufs=1) as resid_pool,
                tc.tile_pool(name="outsb", bufs=4) as out_pool,
                tc.tile_pool(name="dps", bufs=8, space="PSUM") as dense_ps,
            ):
                wd_sb = []
                for fc in range(NHC):
                    t = wd_pool.tile([P, DCOL], F32, name=f"wd_sb{fc}")
                    nc.sync.dma_start(t[:], wdT[fc * P : (fc + 1) * P, :])
                    wd_sb.append(t)
                resid_sb = []
                for ct in range(DCOL // P):
                    t = resid_pool.tile([P, S], F32, name=f"resid{ct}")
                    nc.sync.dma_start(t[:], residT[ct * P : (ct + 1) * P, :])
                    resid_sb.append(t)
                dp = {}
                for ct in range(DCOL // P):
                    for s2 in range(NSC):
                        dp[(ct, s2)] = dense_ps.tile([P, QCH], F32, name="dp")
                for fc in range(NHC):
                    cf = ctxf_pool.tile([P, S], F32, name="cf")
                    nc.sync.dma_start(cf[:], ag_out[fc * P : (fc + 1) * P, :])
                    for ct in range(DCOL // P):
                        for s2 in range(NSC):
                            nc.tensor.matmul(
                                dp[(ct, s2)][:],
                                wd_sb[fc][:, ct * P : (ct + 1) * P],
                                cf[:, s2 * QCH : (s2 + 1) * QCH],
                                start=(fc == 0),
                                stop=(fc == NHC - 1),
                            )
                for ct in range(DCOL // P):
                    for s2 in range(NSC):
                        ot = out_pool.tile([P, QCH], F32, name="ot")
                        nc.scalar.activation(
                            ot[:],
                            dp[(ct, s2)][:],
                            AF.Identity,
                            bias=bdense_sb[:, ct : ct + 1],
                        )
                        nc.vector.tensor_tensor(
                            out=ot[:],
                            in0=ot[:],
                            in1=resid_sb[ct][:, s2 * QCH : (s2 + 1) * QCH],
                            op=ALU.add,
                        )
                        nc.sync.dma_start(
                            outT[ct * P : (ct + 1) * P, s2 * QCH : (s2 + 1) * QCH],
                            ot[:],
                        )
            _free_ag()

    _split_waits(nc)
    return nc


def prepare(hidden_states, residual, alibi, attention_mask, W_qkv, b_qkv, W_dense, b_dense):
    """Host-side input marshalling: slicing per core, zero-FLOP relayouts,
    and mask/alibi analysis for the fixed-shift softmax."""
    inv_norm = 1.0 / math.sqrt(HD)
    hs = np.ascontiguousarray(np.asarray(hidden_states, dtype=np.float32)[0])
    hsT = np.ascontiguousarray(hs.T)
    residT_full = np.ascontiguousarray(np.asarray(residual, dtype=np.float32)[0].T)
    alibi = np.asarray(alibi, dtype=np.float32).reshape(NH, S)
    mask2d = np.asarray(attention_mask).reshape(S, S)  # [q, k], True = masked
    W_qkv = np.asarray(W_qkv, dtype=np.float32)
    b_qkv = np.asarray(b_qkv, dtype=np.float32)
    W_dense = np.asarray(W_dense, dtype=np.float32)
    b_dense = np.asarray(b_dense, dtype=np.float32)

    # block classification on the S.T grid: block (qc, kc) holds
    # k in [kc*128, +128), q in [qc*512, +512)
    block_lists = [[] for _ in range(NQC)]
    mask_tiles = []
    for qc in range(NQC):
        for kc in range(NKC):
            sub = mask2d[qc * QCH : (qc + 1) * QCH, kc * P : (kc + 1) * P]
            if sub.all():
                continue
            if not sub.any():
                block_lists[qc].append((kc, None))
            else:
                mask_tiles.append(
                    np.where(sub.T, np.float32(NEG_BIG), np.float32(0.0))
                )
                block_lists[qc].append((kc, len(mask_tiles) - 1))
    n_part = len(mask_tiles)
    maskadd = (
        np.ascontiguousarray(np.concatenate(mask_tiles, axis=0))
        if n_part
        else None
    )

    # fixed per-q softmax shift: c[h, q] = max over allowed k of alibi[h, k]
    allowed = ~mask2d  # [q, k]
    negc_all = np.zeros((NH, S), dtype=np.float32)
    for h in range(NH):
        masked_vals = np.where(allowed, alibi[h][None, :], -np.inf)
        c = masked_vals.max(axis=1)
        c = np.where(np.isfinite(c), c, 0.0)  # fully-masked rows: degenerate
        negc_all[h] = -c

    in_maps = []
    for core in range(N_CORES):
        heads = [NH_LOC * core + i for i in range(NH_LOC)]
        rows = []
        brows = []
        for h in heads:
            for three in range(3):
                sl = slice(h * 3 * HD + three * HD, h * 3 * HD + (three + 1) * HD)
                w = W_qkv[sl].copy()
                bb_ = b_qkv[sl].copy()
                if three == 0:  # fold 1/sqrt(hd) into the Q projection
                    w *= inv_norm
                    bb_ *= inv_norm
                rows.append(w)
                brows.append(bb_)
        w_sel = np.concatenate(rows, axis=0)  # [768, 2048]
        b_sel = np.concatenate(brows, axis=0)  # [768]
        wqkvT = np.ascontiguousarray(w_sel.T)  # [2048, 768]
        bqkv_c = np.ascontiguousarray(b_sel.reshape(OPC, P).T)  # [128, 6]
        alibi_c = np.ascontiguousarray(
            np.concatenate(
                [alibi[h].reshape(NKC, P).T for h in heads], axis=1
            )
        )  # [128, 32]: col h_loc*16+kc
        negc_c = np.ascontiguousarray(negc_all[heads])  # [2, 2048]
        wdT_c = np.ascontiguousarray(
            W_dense[core * DCOL : (core + 1) * DCOL, :].T
        )  # [2048, 256]
        bdense_c = np.ascontiguousarray(
            b_dense[core * DCOL : (core + 1) * DCOL].reshape(DCOL // P, P).T
        )  # [128, 2]
        residT_c = np.ascontiguousarray(
            residT_full[core * DCOL : (core + 1) * DCOL, :]
        )  # [256, 2048]
        m = {
            "hsT": hsT,
            "wqkvT": wqkvT,
            "bqkv": bqkv_c,
            "alibi_b": alibi_c,
            "negc": negc_c,
            "wdT": wdT_c,
            "bdense": bdense_c,
            "residT": residT_c,
            "identity": np.eye(P, dtype=np.float32),
            "ones128": np.ones((P, P), dtype=np.float32),
        }
        if n_part:
            m["maskadd"] = maskadd
        in_maps.append(m)
    return block_lists, n_part, in_maps


def assemble(results):
    shards = [results[c]["outT"] for c in range(N_CORES)]  # [256, 2048] each
    outT = np.concatenate(shards, axis=0)  # [2048 cols, 2048 s]
    return np.ascontiguousarray(outT.T).reshape(B, S, H)


_cache = {}


def kernel(**inputs) -> np.ndarray:
    block_lists, n_part, in_maps = prepare(**inputs)
    key = (tuple(tuple(bl) for bl in block_lists), n_part)
    if key not in _cache:
        _cache[key] = build_program(block_lists, n_part)
    nc = _cache[key]
    res = run_bass_kernel_spmd(nc, in_maps, list(range(N_CORES)), trace=False)
    return assemble(res.results)


# revision 14
# speedup vs baseline: 4085.1789x; 4085.1789x over previous
"""BLOOM attention block (QKV proj + alibi causal attention + dense + residual)
as a head-sharded (tensor-parallel) Bass kernel on 8 Trainium2 NeuronCores.

Sharding: 2 heads per core. Per core:
  - qkvT[o, s] = Wsel @ hs.T computed from host-pretransposed hs.T (16 MB,
    replicated) and the core's 768-row slice of W_qkv (transposed, Q rows
    pre-scaled by 1/sqrt(hd)).
  - Attention in transposed orientation: S.T[k, q] = K @ Q.T so the exp output
    is already P.T, which feeds the PV matmul with V (PE-transposed per head)
    as the stationary operand. Softmax uses a fixed per-q shift
    c[q] = max_{allowed k} alibi[k] (exact by shift invariance; no max pass):
    P.T = exp(S.T + alibi[k] - c[q]), l[q] = ones @ P.T (replicated rows),
    ctx.T[d, q] = (V.T)·P.T / l.
  - ctx.T [256, 2048] per core is AllGathered (2 MB/rank) into the full
    [2048, 2048] ctx.T; each core then computes a 256-column shard of the
    dense output as out.T[col, s] = WdT.T @ ctx.T + b + residual.T.
Host assembles the 8 column shards.
"""

import sys

sys.path.insert(0, "/opt/trn_rl_repo")

import math

import numpy as np

import concourse.bass as bass
import concourse.mybir as mybir
import concourse.tile as tile
from concourse.bass_utils import run_bass_kernel_spmd

F32 = mybir.dt.float32
F32R = mybir.dt.float32r
AF = mybir.ActivationFunctionType
ALU = mybir.AluOpType

B, S, H, NH = 1, 2048, 2048, 16
HD = H // NH  # 128
N_CORES = 8
NH_LOC = NH // N_CORES  # 2 heads per core
OPC = 3 * NH_LOC  # 6 qkv output row-blocks of 128 per core
P = 128
QCH = 512  # q chunk (free dim) for attention blocks
NQC = S // QCH  # 4
NKC = S // P  # 16
NSC = S // QCH  # 4  s-chunks in qkv projection
NHC = H // P  # 16  contraction chunks
DCOL = H // N_CORES  # 256 dense output columns per core
NEG_BIG = -1.0e38

_ctr = [0]


def _split_waits(nc, default_limit=1, drain_limit=1):
    """This container's walrus accepts few sync-wait commands per instruction
    (1 for CTRL/Drain and some others), while Tile attaches one wait per
    upstream proc. Hoist the excess waits onto standalone EventSemaphore
    instructions just before the over-subscribed instruction on the same
    engine (same sequencer => identical blocking semantics)."""
    for bb in nc.main_func.blocks:
        new = []
        changed = False
        for ins in bb.instructions:
            si = ins.sync_info
            ow = list(si.on_wait) if si is not None else []
            lim = drain_limit if isinstance(ins, mybir.InstDrain) else default_limit
            if len(ow) > lim:
                for w in ow[:-lim]:
                    _ctr[0] += 1
                    nop = mybir.InstEventSemaphore(
                        name=f"I-waitsplit-{_ctr[0]}",
                        engine=ins.engine,
                        ins=[],
                        outs=[],
                        sync_info=mybir.SyncInfo(on_wait=[w], on_update=[]),
                    )
                    nc.register_instruction(nop)
                    new.append(nop)
                    changed = True
                ins.sync_info = mybir.SyncInfo(
                    on_wait=ow[-lim:], on_update=list(si.on_update)
                )
            new.append(ins)
        if changed:
            bb.instructions = new


def build_program(block_lists, n_part, n_iters=1, upto=4, fast_mm=True):
    """block_lists: per qc, list of (kc, mask_tile_idx_or_None), shared by all
    cores/heads (the mask input is head-independent). n_part: number of
    partial-block mask tiles staged in the maskadd input. n_iters repeats the
    whole computation in one NEFF (for on-device timing via deltas).
    fast_mm: use float32r matmul operands (1 cycle/row on the PE vs 4 for
    fp32; ~tf32-like operand rounding, fp32 accumulate)."""
    nc = bass.Bass()
    MDT = F32R if fast_mm else F32

    hsT = nc.dram_tensor("hsT", [H, S], MDT, kind="ExternalInput")
    wqkvT = nc.dram_tensor("wqkvT", [H, OPC * P], MDT, kind="ExternalInput")
    bqkv = nc.dram_tensor("bqkv", [P, OPC], F32, kind="ExternalInput")
    alibi_b = nc.dram_tensor("alibi_b", [P, NH_LOC * NKC], F32, kind="ExternalInput")
    negc = nc.dram_tensor("negc", [NH_LOC, S], F32, kind="ExternalInput")
    wdT = nc.dram_tensor("wdT", [H, DCOL], MDT, kind="ExternalInput")
    bdense = nc.dram_tensor("bdense", [P, DCOL // P], F32, kind="ExternalInput")
    residT = nc.dram_tensor("residT", [DCOL, S], F32, kind="ExternalInput")
    identity = nc.dram_tensor("identity", [P, P], MDT, kind="ExternalInput")
    ones128 = nc.dram_tensor("ones128", [P, P], MDT, kind="ExternalInput")
    maskadd = (
        nc.dram_tensor("maskadd", [n_part * P, QCH], F32, kind="ExternalInput")
        if n_part
        else None
    )
    outT = nc.dram_tensor("outT", [DCOL, S], F32, kind="ExternalOutput")

    with tile.TileContext(nc) as tc:
        with (
            tc.tile_pool(name="consts", bufs=1) as consts,
            tc.tile_pool(name="qkvsb", bufs=1) as qkvsb_pool,
            tc.tile_pool(name="dram", bufs=1, space="DRAM") as dram_pool,
        ):
            ident_sb = consts.tile([P, P], MDT)
            nc.gpsimd.dma_start(ident_sb[:], identity[:])
            ones_sb = consts.tile([P, P], MDT)
            nc.gpsimd.dma_start(ones_sb[:], ones128[:])
            bqkv_sb = consts.tile([P, OPC], F32)
            nc.gpsimd.dma_start(bqkv_sb[:], bqkv[:])
            alibi_sb = consts.tile([P, NH_LOC * NKC], F32)
            nc.gpsimd.dma_start(alibi_sb[:], alibi_b[:])
            bdense_sb = consts.tile([P, DCOL // P], F32)
            nc.gpsimd.dma_start(bdense_sb[:], bdense[:])
            # -c[h, q] broadcast to all partitions (DMA stride-0 read)
            negc_sb = []
            for h in range(NH_LOC):
                t = consts.tile([P, S], F32, name=f"negc_sb{h}")
                nc.gpsimd.dma_start(t[:], negc[h : h + 1, :].to_broadcast((P, S)))
                negc_sb.append(t)

            # qkvT[o, s]: 6 row-blocks [128, 2048]
            qkv_sb = [
                qkvsb_pool.tile([P, S], MDT, name=f"qkv_sb{ot}") for ot in range(OPC)
            ]
            ag_in = dram_pool.tile([NH_LOC * P, S], MDT, name="ag_in")
            ag_out, free_ag = tc.tile(
                [H, S], MDT, space="DRAM", addr_space="Shared", name="ag_out"
            )

            for _it in range(n_iters):
                _emit_iteration(
                    nc, tc, block_lists, n_part,
                    hsT, wqkvT, wdT, residT, maskadd, outT,
                    ident_sb, ones_sb, bqkv_sb, alibi_sb, bdense_sb, negc_sb,
                    qkv_sb, ag_in, ag_out, upto, MDT,
                )
            free_ag()

    _split_waits(nc)
    return nc


def _emit_iteration(
    nc, tc, block_lists, n_part,
    hsT, wqkvT, wdT, residT, maskadd, outT,
    ident_sb, ones_sb, bqkv_sb, alibi_sb, bdense_sb, negc_sb,
    qkv_sb, ag_in, ag_out, upto=4, MDT=F32R,
):
    # ---- Phase 1: fused QKV projection (contraction over H) ----
    with (
        tc.tile_pool(name="wq", bufs=1) as wq_pool,
        tc.tile_pool(name="hst", bufs=2) as hst_pool,
        tc.tile_pool(name="qkvps", bufs=3, space="PSUM") as qkv_ps,
    ):
        # all 16 [128, 768] weight chunks in one 6 MB DMA
        wq_sb = wq_pool.tile([P, NHC, OPC * P], MDT, name="wq_sb")
        nc.gpsimd.dma_start(
            wq_sb[:], wqkvT[:].rearrange("(c p) o -> p c o", p=P)
        )
        for sc in range(NSC):
            s0 = sc * QCH
            # all 16 [128, 512] hs.T chunks for this s-slab in one 4 MB DMA
            hs_t = hst_pool.tile([P, NHC, QCH], MDT, name="hs_t")
            nc.sync.dma_start(
                hs_t[:], hsT[:, s0 : s0 + QCH].rearrange("(c p) s -> p c s", p=P)
            )
            for ot in range(OPC):
                ps = qkv_ps.tile([P, QCH], F32, name="qkv_acc")
                for hc in range(NHC):
                    nc.tensor.matmul(
                        ps[:],
                        wq_sb[:, hc, ot * P : (ot + 1) * P],
                        hs_t[:, hc, :],
                        start=(hc == 0),
                        stop=(hc == NHC - 1),
                    )
                nc.scalar.activation(
                    qkv_sb[ot][:, s0 : s0 + QCH],
                    ps[:],
                    AF.Identity,
                    bias=bqkv_sb[:, ot : ot + 1],
                )

    if upto < 2:
        return
    # ---- Phase 2: attention per head ----
    with (
        tc.tile_pool(name="ctxtsb", bufs=1) as ctxt_pool,
        tc.tile_pool(name="masks", bufs=1) as mask_pool,
        tc.tile_pool(name="vnat", bufs=1) as vnat_pool,
        tc.tile_pool(name="pt", bufs=16) as pt_pool,
        tc.tile_pool(name="lrec", bufs=2) as lrec_pool,
        tc.tile_pool(name="vtps", bufs=2, space="PSUM") as vt_ps,
        tc.tile_pool(name="stps", bufs=2, space="PSUM") as st_ps,
        tc.tile_pool(name="ctxps", bufs=2, space="PSUM") as ctx_ps,
        tc.tile_pool(name="lps", bufs=2, space="PSUM") as l_ps,
    ):
        ctxT_sb = [
            ctxt_pool.tile([P, S], MDT, name=f"ctxT_sb{h}") for h in range(NH_LOC)
        ]
        mask_big = None
        if n_part:
            mask_big = mask_pool.tile([P, n_part, QCH], F32, name="mask_big")
            nc.gpsimd.dma_start(
                mask_big[:], maskadd[:].rearrange("(c p) s -> p c s", p=P)
            )
        for h in range(NH_LOC):
            QT = qkv_sb[3 * h + 0]
            KT = qkv_sb[3 * h + 1]
            VT = qkv_sb[3 * h + 2]
            # V natural [k, d] via PE transpose of VT column blocks
            vn = []
            for kc in range(NKC):
                vp = vt_ps.tile([P, P], MDT, name="vt_p")
                nc.tensor.transpose(vp[:], VT[:, kc * P : (kc + 1) * P], ident_sb[:])
                t = vnat_pool.tile([P, P], MDT, name=f"vn{h}_{kc}")
                nc.vector.tensor_copy(t[:], vp[:])
                vn.append(t)
            for qc in range(NQC):
                q0 = qc * QCH
                kcs = block_lists[qc]
                pts = {}
                for kc, mi in kcs:
                    st = st_ps.tile([P, QCH], F32, name="st")
                    nc.tensor.matmul(
                        st[:],
                        KT[:, kc * P : (kc + 1) * P],
                        QT[:, q0 : q0 + QCH],
                        start=True,
                        stop=True,
                    )
                    nc.vector.tensor_tensor(
                        out=st[:],
                        in0=st[:],
                        in1=negc_sb[h][:, q0 : q0 + QCH],
                        op=ALU.add,
                    )
                    if mi is not None:
                        nc.vector.tensor_tensor(
                            out=st[:], in0=st[:], in1=mask_big[:, mi, :], op=ALU.add
                        )
                    pt = pt_pool.tile([P, QCH], MDT, name="pt")
                    col = h * NKC + kc
                    nc.scalar.activation(
                        pt[:], st[:], AF.Exp, bias=alibi_sb[:, col : col + 1]
                    )
                    pts[kc] = pt
                cps = ctx_ps.tile([P, QCH], F32, name="cacc")
                for i, (kc, _mi) in enumerate(kcs):
                    nc.tensor.matmul(
                        cps[:],
                        vn[kc][:],
                        pts[kc][:],
                        start=(i == 0),
                        stop=(i == len(kcs) - 1),
                    )
                lps = l_ps.tile([P, QCH], F32, name="lacc")
                for i, (kc, _mi) in enumerate(kcs):
                    nc.tensor.matmul(
                        lps[:],
                        ones_sb[:],
                        pts[kc][:],
                        start=(i == 0),
                        stop=(i == len(kcs) - 1),
                    )
                rec = lrec_pool.tile([P, QCH], F32, name="rec")
                nc.vector.reciprocal(rec[:], lps[:])
                nc.vector.tensor_tensor(
                    out=ctxT_sb[h][:, q0 : q0 + QCH],
                    in0=cps[:],
                    in1=rec[:],
                    op=ALU.mult,
                )

        if upto >= 3:
            # ---- Phase 3: AllGather ctx.T across cores ----
            for h in range(NH_LOC):
                nc.gpsimd.dma_start(ag_in[h * P : (h + 1) * P, :], ctxT_sb[h][:])
    if upto < 3:
        return
    nc.gpsimd.collective_compute(
        "AllGather",
        ALU.bypass,
        replica_groups=[list(range(N_CORES))],
        ins=[ag_in.opt()],
        outs=[ag_out.opt()],
    )

    if upto < 4:
        return
    # ---- Phase 4: dense column shard + bias + residual ----
    with (
        tc.tile_pool(name="wd", bufs=1) as wd_pool,
        tc.tile_pool(name="ctxf", bufs=3) as ctxf_pool,
        tc.tile_pool(name="residsb", bufs=1) as resid_pool,
        tc.tile_pool(name="outsb", bufs=4) as out_pool,
        tc.tile_pool(name="dps", bufs=8, space="PSUM") as dense_ps,
    ):
        wd_sb = wd_pool.tile([P, NHC, DCOL], MDT, name="wd_sb")
        nc.scalar.dma_start(wd_sb[:], wdT[:].rearrange("(c p) o -> p c o", p=P))
        resid_sb = []
        for ct in range(DCOL // P):
            t = resid_pool.tile([P, S], F32, name=f"resid{ct}")
            nc.scalar.dma_start(t[:], residT[ct * P : (ct + 1) * P, :])
            resid_sb.append(t)
        dp = {}
        for ct in range(DCOL // P):
            for s2 in range(NSC):
                dp[(ct, s2)] = dense_ps.tile([P, QCH], F32, name="dp")
        for fc in range(NHC):
            cf = ctxf_pool.tile([P, S], MDT, name="cf")
            eng = nc.sync if fc % 2 == 0 else nc.scalar
            eng.dma_start(cf[:], ag_out[fc * P : (fc + 1) * P, :])
            for ct in range(DCOL // P):
                for s2 in range(NSC):
                    nc.tensor.matmul(
                        dp[(ct, s2)][:],
                        wd_sb[:, fc, ct * P : (ct + 1) * P],
                        cf[:, s2 * QCH : (s2 + 1) * QCH],
                        start=(fc == 0),
                        stop=(fc == NHC - 1),
                    )
        for ct in range(DCOL // P):
            for s2 in range(NSC):
                ot = out_pool.tile([P, QCH], F32, name="ot")
                nc.scalar.activation(
                    ot[:], dp[(ct, s2)][:], AF.Identity, bias=bdense_sb[:, ct : ct + 1]
                )
                nc.vector.tensor_tensor(
                    out=ot[:],
                    in0=ot[:],
                    in1=resid_sb[ct][:, s2 * QCH : (s2 + 1) * QCH],
                    op=ALU.add,
                )
                nc.sync.dma_start(
                    outT[ct * P : (ct + 1) * P, s2 * QCH : (s2 + 1) * QCH], ot[:]
                )


def prepare(hidden_states, residual, alibi, attention_mask, W_qkv, b_qkv, W_dense, b_dense):
    """Host-side input marshalling: slicing per core, zero-FLOP relayouts,
    and mask/alibi analysis for the fixed-shift softmax."""
    inv_norm = 1.0 / math.sqrt(HD)
    hs = np.ascontiguousarray(np.asarray(hidden_states, dtype=np.float32)[0])
    hsT = np.ascontiguousarray(hs.T)
    residT_full = np.ascontiguousarray(np.asarray(residual, dtype=np.float32)[0].T)
    alibi = np.asarray(alibi, dtype=np.float32).reshape(NH, S)
    mask2d = np.asarray(attention_mask).reshape(S, S)  # [q, k], True = masked
    W_qkv = np.asarray(W_qkv, dtype=np.float32)
    b_qkv = np.asarray(b_qkv, dtype=np.float32)
    W_dense = np.asarray(W_dense, dtype=np.float32)
    b_dense = np.asarray(b_dense, dtype=np.float32)

    # block classification on the S.T grid: block (qc, kc) holds
    # k in [kc*128, +128), q in [qc*512, +512)
    block_lists = [[] for _ in range(NQC)]
    mask_tiles = []
    for qc in range(NQC):
        for kc in range(NKC):
            sub = mask2d[qc * QCH : (qc + 1) * QCH, kc * P : (kc + 1) * P]
            if sub.all():
                continue
            if not sub.any():
                block_lists[qc].append((kc, None))
            else:
                mask_tiles.append(
                    np.where(sub.T, np.float32(NEG_BIG), np.float32(0.0))
                )
                block_lists[qc].append((kc, len(mask_tiles) - 1))
    n_part = len(mask_tiles)
    maskadd = (
        np.ascontiguousarray(np.concatenate(mask_tiles, axis=0)) if n_part else None
    )

    # fixed per-q softmax shift: c[h, q] = max over allowed k of alibi[h, k]
    allowed = ~mask2d  # [q, k]
    negc_all = np.zeros((NH, S), dtype=np.float32)
    for h in range(NH):
        masked_vals = np.where(allowed, alibi[h][None, :], -np.inf)
        c = masked_vals.max(axis=1)
        c = np.where(np.isfinite(c), c, 0.0)  # fully-masked rows: degenerate
        negc_all[h] = -c

    in_maps = []
    for core in range(N_CORES):
        heads = [NH_LOC * core + i for i in range(NH_LOC)]
        rows = []
        brows = []
        for h in heads:
            for three in range(3):
                sl = slice(h * 3 * HD + three * HD, h * 3 * HD + (three + 1) * HD)
                w = W_qkv[sl].copy()
                bb_ = b_qkv[sl].copy()
                if three == 0:  # fold 1/sqrt(hd) into the Q projection
                    w *= inv_norm
                    bb_ *= inv_norm
                rows.append(w)
                brows.append(bb_)
        w_sel = np.concatenate(rows, axis=0)  # [768, 2048]
        b_sel = np.concatenate(brows, axis=0)  # [768]
        wqkvT = np.ascontiguousarray(w_sel.T)  # [2048, 768]
        bqkv_c = np.ascontiguousarray(b_sel.reshape(OPC, P).T)  # [128, 6]
        alibi_c = np.ascontiguousarray(
            np.concatenate([alibi[h].reshape(NKC, P).T for h in heads], axis=1)
        )  # [128, 32]: col h_loc*16+kc
        negc_c = np.ascontiguousarray(negc_all[heads])  # [2, 2048]
        wdT_c = np.ascontiguousarray(
            W_dense[core * DCOL : (core + 1) * DCOL, :].T
        )  # [2048, 256]
        bdense_c = np.ascontiguousarray(
            b_dense[core * DCOL : (core + 1) * DCOL].reshape(DCOL // P, P).T
        )  # [128, 2]
        residT_c = np.ascontiguousarray(
            residT_full[core * DCOL : (core + 1) * DCOL, :]
        )  # [256, 2048]
        m = {
            "hsT": hsT,
            "wqkvT": wqkvT,
            "bqkv": bqkv_c,
            "alibi_b": alibi_c,
            "negc": negc_c,
            "wdT": wdT_c,
            "bdense": bdense_c,
            "residT": residT_c,
            "identity": np.eye(P, dtype=np.float32),
            "ones128": np.ones((P, P), dtype=np.float32),
        }
        if n_part:
            m["maskadd"] = maskadd
        in_maps.append(m)
    return block_lists, n_part, in_maps


def assemble(results):
    shards = [results[c]["outT"] for c in range(N_CORES)]  # [256, 2048] each
    outT = np.concatenate(shards, axis=0)  # [2048 cols, 2048 s]
    return np.ascontiguousarray(outT.T).reshape(B, S, H)


_cache = {}


def kernel(**inputs) -> np.ndarray:
    block_lists, n_part, in_maps = prepare(**inputs)
    key = (tuple(tuple(bl) for bl in block_lists), n_part)
    if key not in _cache:
        _cache[key] = build_program(block_lists, n_part)
    nc = _cache[key]
    res = run_bass_kernel_spmd(nc, in_maps, list(range(N_CORES)), trace=False)
    return assemble(res.results)


# revision 16
# speedup vs baseline: 19057.1873x; 4.6650x over previous
"""BLOOM attention block (QKV proj + alibi causal attention + dense + residual)
as a head-sharded (tensor-parallel) Bass kernel on 8 Trainium2 NeuronCores.

Sharding: 2 heads per core. Per core:
  - qkvT[o, s] = Wsel @ hs.T computed from host-pretransposed hs.T (16 MB,
    replicated) and the core's 768-row slice of W_qkv (transposed, Q rows
    pre-scaled by 1/sqrt(hd)).
  - Attention in transposed orientation: S.T[k, q] = K @ Q.T so the exp output
    is already P.T, which feeds the PV matmul with V (PE-transposed per head)
    as the stationary operand. Softmax uses a fixed per-q shift
    c[q] = max_{allowed k} alibi[k] (exact by shift invariance; no max pass):
    P.T = exp(S.T + alibi[k] - c[q]), l[q] = ones @ P.T (replicated rows),
    ctx.T[d, q] = (V.T)·P.T / l.
  - ctx.T [256, 2048] per core is AllGathered (2 MB/rank) into the full
    [2048, 2048] ctx.T; each core then computes a 256-column shard of the
    dense output as out.T[col, s] = WdT.T @ ctx.T + b + residual.T.
Host assembles the 8 column shards.
"""

import sys

sys.path.insert(0, "/opt/trn_rl_repo")

import math

import numpy as np

import concourse.bass as bass
import concourse.mybir as mybir
import concourse.tile as tile
from concourse.bass_utils import run_bass_kernel_spmd

F32 = mybir.dt.float32
F32R = mybir.dt.float32r
AF = mybir.ActivationFunctionType
ALU = mybir.AluOpType

B, S, H, NH = 1, 2048, 2048, 16
HD = H // NH  # 128
N_CORES = 8
NH_LOC = NH // N_CORES  # 2 heads per core
OPC = 3 * NH_LOC  # 6 qkv output row-blocks of 128 per core
P = 128
QCH = 512  # q chunk (free dim) for attention blocks
NQC = S // QCH  # 4
NKC = S // P  # 16
NSC = S // QCH  # 4  s-chunks in qkv projection
NHC = H // P  # 16  contraction chunks
DCOL = H // N_CORES  # 256 dense output columns per core
NEG_BIG = -1.0e38

_ctr = [0]


def _split_waits(nc, default_limit=1, drain_limit=1):
    """This container's walrus accepts few sync-wait commands per instruction
    (1 for CTRL/Drain and some others), while Tile attaches one wait per
    upstream proc. Hoist the excess waits onto standalone EventSemaphore
    instructions just before the over-subscribed instruction on the same
    engine (same sequencer => identical blocking semantics)."""
    for bb in nc.main_func.blocks:
        new = []
        changed = False
        for ins in bb.instructions:
            si = ins.sync_info
            ow = list(si.on_wait) if si is not None else []
            lim = drain_limit if isinstance(ins, mybir.InstDrain) else default_limit
            if len(ow) > lim:
                for w in ow[:-lim]:
                    _ctr[0] += 1
                    nop = mybir.InstEventSemaphore(
                        name=f"I-waitsplit-{_ctr[0]}",
                        engine=ins.engine,
                        ins=[],
                        outs=[],
                        sync_info=mybir.SyncInfo(on_wait=[w], on_update=[]),
                    )
                    nc.register_instruction(nop)
                    new.append(nop)
                    changed = True
                ins.sync_info = mybir.SyncInfo(
                    on_wait=ow[-lim:], on_update=list(si.on_update)
                )
            new.append(ins)
        if changed:
            bb.instructions = new


def build_program(block_lists, n_part, n_iters=1, upto=4, fast_mm=True):
    """block_lists: per qc, list of (kc, mask_tile_idx_or_None), shared by all
    cores/heads (the mask input is head-independent). n_part: number of
    partial-block mask tiles staged in the maskadd input. n_iters repeats the
    whole computation in one NEFF (for on-device timing via deltas).
    fast_mm: use float32r matmul operands (1 cycle/row on the PE vs 4 for
    fp32; ~tf32-like operand rounding, fp32 accumulate)."""
    nc = bass.Bass()
    MDT = F32R if fast_mm else F32

    hsT = nc.dram_tensor("hsT", [H, S], MDT, kind="ExternalInput")
    wqkvT = nc.dram_tensor("wqkvT", [H, OPC * P], MDT, kind="ExternalInput")
    bqkv = nc.dram_tensor("bqkv", [P, OPC], F32, kind="ExternalInput")
    alibi_b = nc.dram_tensor("alibi_b", [P, NH_LOC * NKC], F32, kind="ExternalInput")
    negc = nc.dram_tensor("negc", [NH_LOC, S], F32, kind="ExternalInput")
    wdT = nc.dram_tensor("wdT", [H, DCOL], MDT, kind="ExternalInput")
    bdense = nc.dram_tensor("bdense", [P, DCOL // P], F32, kind="ExternalInput")
    residT = nc.dram_tensor("residT", [DCOL, S], F32, kind="ExternalInput")
    identity = nc.dram_tensor("identity", [P, P], MDT, kind="ExternalInput")
    ones128 = nc.dram_tensor("ones128", [P, P], MDT, kind="ExternalInput")
    maskadd = (
        nc.dram_tensor("maskadd", [n_part * P, QCH], F32, kind="ExternalInput")
        if n_part
        else None
    )
    outT = nc.dram_tensor("outT", [DCOL, S], F32, kind="ExternalOutput")

    with tile.TileContext(nc) as tc:
        with (
            tc.tile_pool(name="consts", bufs=1) as consts,
            tc.tile_pool(name="qkvsb", bufs=1) as qkvsb_pool,
            tc.tile_pool(name="dram", bufs=1, space="DRAM") as dram_pool,
        ):
            ident_sb = consts.tile([P, P], MDT)
            nc.gpsimd.dma_start(ident_sb[:], identity[:])
            ones_sb = consts.tile([P, P], MDT)
            nc.gpsimd.dma_start(ones_sb[:], ones128[:])
            bqkv_sb = consts.tile([P, OPC], F32)
            nc.gpsimd.dma_start(bqkv_sb[:], bqkv[:])
            alibi_sb = consts.tile([P, NH_LOC * NKC], F32)
            nc.gpsimd.dma_start(alibi_sb[:], alibi_b[:])
            bdense_sb = consts.tile([P, DCOL // P], F32)
            nc.gpsimd.dma_start(bdense_sb[:], bdense[:])
            # -c[h, q] broadcast to all partitions (DMA stride-0 read)
            negc_sb = []
            for h in range(NH_LOC):
                t = consts.tile([P, S], F32, name=f"negc_sb{h}")
                nc.gpsimd.dma_start(t[:], negc[h : h + 1, :].to_broadcast((P, S)))
                negc_sb.append(t)

            # qkvT[o, s]: 6 row-blocks [128, 2048]
            qkv_sb = [
                qkvsb_pool.tile([P, S], MDT, name=f"qkv_sb{ot}") for ot in range(OPC)
            ]
            ag_in = [
                dram_pool.tile([P, S], MDT, name=f"ag_in{h}") for h in range(NH_LOC)
            ]
            ag_out = []
            free_ags = []
            for h in range(NH_LOC):
                t, fr = tc.tile(
                    [N_CORES * P, S], MDT, space="DRAM", addr_space="Shared",
                    name=f"ag_out{h}",
                )
                ag_out.append(t)
                free_ags.append(fr)

            for _it in range(n_iters):
                _emit_iteration(
                    nc, tc, block_lists, n_part,
                    hsT, wqkvT, wdT, residT, maskadd, outT,
                    ident_sb, ones_sb, bqkv_sb, alibi_sb, bdense_sb, negc_sb,
                    qkv_sb, ag_in, ag_out, upto, MDT,
                )
            for fr in free_ags:
                fr()

    _split_waits(nc)
    return nc


def _emit_iteration(
    nc, tc, block_lists, n_part,
    hsT, wqkvT, wdT, residT, maskadd, outT,
    ident_sb, ones_sb, bqkv_sb, alibi_sb, bdense_sb, negc_sb,
    qkv_sb, ag_in, ag_out, upto=4, MDT=F32R,
):
    # ---- Phase 1: fused QKV projection (contraction over H) ----
    with (
        tc.tile_pool(name="wq", bufs=1) as wq_pool,
        tc.tile_pool(name="hst", bufs=2) as hst_pool,
        tc.tile_pool(name="qkvps", bufs=3, space="PSUM") as qkv_ps,
    ):
        # 16 [128, 768] weight chunks, 4 sub-DMAs so the first matmuls
        # can start before the whole 6 MB lands
        wq_sb = wq_pool.tile([P, NHC, OPC * P], MDT, name="wq_sb")
        for j in range(4):
            nc.gpsimd.dma_start(
                wq_sb[:, 4 * j : 4 * (j + 1), :],
                wqkvT[4 * j * P : 4 * (j + 1) * P, :].rearrange(
                    "(c p) o -> p c o", p=P
                ),
            )
        for sc in range(NSC):
            s0 = sc * QCH
            # 16 [128, 512] hs.T chunks for this s-slab, 4 sub-DMAs
            hs_t = hst_pool.tile([P, NHC, QCH], MDT, name="hs_t")
            for j in range(4):
                nc.sync.dma_start(
                    hs_t[:, 4 * j : 4 * (j + 1), :],
                    hsT[4 * j * P : 4 * (j + 1) * P, s0 : s0 + QCH].rearrange(
                        "(c p) s -> p c s", p=P
                    ),
                )
            for ot in range(OPC):
                ps = qkv_ps.tile([P, QCH], F32, name="qkv_acc")
                for hc in range(NHC):
                    nc.tensor.matmul(
                        ps[:],
                        wq_sb[:, hc, ot * P : (ot + 1) * P],
                        hs_t[:, hc, :],
                        start=(hc == 0),
                        stop=(hc == NHC - 1),
                    )
                nc.scalar.activation(
                    qkv_sb[ot][:, s0 : s0 + QCH],
                    ps[:],
                    AF.Identity,
                    bias=bqkv_sb[:, ot : ot + 1],
                )

    if upto < 2:
        return
    # ---- Phase 2: attention per head ----
    with (
        tc.tile_pool(name="ctxtsb", bufs=1) as ctxt_pool,
        tc.tile_pool(name="masks", bufs=1) as mask_pool,
        tc.tile_pool(name="vnat", bufs=1) as vnat_pool,
        tc.tile_pool(name="pt", bufs=20) as pt_pool,
        tc.tile_pool(name="lrec", bufs=2) as lrec_pool,
        tc.tile_pool(name="vtps", bufs=1, space="PSUM") as vt_ps,
        tc.tile_pool(name="stps", bufs=3, space="PSUM") as st_ps,
        tc.tile_pool(name="ctxps", bufs=2, space="PSUM") as ctx_ps,
        tc.tile_pool(name="lps", bufs=2, space="PSUM") as l_ps,
    ):
        ctxT_sb = [
            ctxt_pool.tile([P, S], MDT, name=f"ctxT_sb{h}") for h in range(NH_LOC)
        ]
        mask_big = None
        if n_part:
            mask_big = mask_pool.tile([P, n_part, QCH], F32, name="mask_big")
            nc.gpsimd.dma_start(
                mask_big[:], maskadd[:].rearrange("(c p) s -> p c s", p=P)
            )
        for h in range(NH_LOC):
            QT = qkv_sb[3 * h + 0]
            KT = qkv_sb[3 * h + 1]
            VT = qkv_sb[3 * h + 2]
            # V natural [k, d] via PE transpose of VT column blocks
            vn = []
            for kc in range(NKC):
                vp = vt_ps.tile([P, P], MDT, name="vt_p")
                nc.tensor.transpose(vp[:], VT[:, kc * P : (kc + 1) * P], ident_sb[:])
                t = vnat_pool.tile([P, P], MDT, name=f"vn{h}_{kc}")
                nc.vector.tensor_copy(t[:], vp[:])
                vn.append(t)
            for qc in range(NQC):
                q0 = qc * QCH
                kcs = block_lists[qc]
                pts = {}
                for kc, mi in kcs:
                    st = st_ps.tile([P, QCH], F32, name="st")
                    nc.tensor.matmul(
                        st[:],
                        KT[:, kc * P : (kc + 1) * P],
                        QT[:, q0 : q0 + QCH],
                        start=True,
                        stop=True,
                    )
                    nc.vector.tensor_tensor(
                        out=st[:],
                        in0=st[:],
                        in1=negc_sb[h][:, q0 : q0 + QCH],
                        op=ALU.add,
                    )
                    if mi is not None:
                        nc.vector.tensor_tensor(
                            out=st[:], in0=st[:], in1=mask_big[:, mi, :], op=ALU.add
                        )
                    pt = pt_pool.tile([P, QCH], MDT, name="pt")
                    col = h * NKC + kc
                    nc.scalar.activation(
                        pt[:], st[:], AF.Exp, bias=alibi_sb[:, col : col + 1]
                    )
                    pts[kc] = pt
                cps = ctx_ps.tile([P, QCH], F32, name="cacc")
                for i, (kc, _mi) in enumerate(kcs):
                    nc.tensor.matmul(
                        cps[:],
                        vn[kc][:],
                        pts[kc][:],
                        start=(i == 0),
                        stop=(i == len(kcs) - 1),
                    )
                lps = l_ps.tile([P, QCH], F32, name="lacc")
                for i, (kc, _mi) in enumerate(kcs):
                    nc.tensor.matmul(
                        lps[:],
                        ones_sb[:],
                        pts[kc][:],
                        start=(i == 0),
                        stop=(i == len(kcs) - 1),
                    )
                rec = lrec_pool.tile([P, QCH], F32, name="rec")
                nc.vector.reciprocal(rec[:], lps[:])
                nc.vector.tensor_tensor(
                    out=ctxT_sb[h][:, q0 : q0 + QCH],
                    in0=cps[:],
                    in1=rec[:],
                    op=ALU.mult,
                )

        if upto >= 3:
            # ---- Phase 3: per-head AllGather, launched as soon as each
            # head's ctx.T is ready so AG(h=0) hides under head 1's attention
            for h in range(NH_LOC):
                nc.gpsimd.dma_start(ag_in[h][:], ctxT_sb[h][:])
                nc.gpsimd.collective_compute(
                    "AllGather",
                    ALU.bypass,
                    replica_groups=[list(range(N_CORES))],
                    ins=[ag_in[h].opt()],
                    outs=[ag_out[h].opt()],
                )
    if upto < 3:
        return

    if upto < 4:
        return
    # ---- Phase 4: dense column shard + bias + residual ----
    with (
        tc.tile_pool(name="wd", bufs=1) as wd_pool,
        tc.tile_pool(name="ctxf", bufs=3) as ctxf_pool,
        tc.tile_pool(name="residsb", bufs=1) as resid_pool,
        tc.tile_pool(name="outsb", bufs=4) as out_pool,
        tc.tile_pool(name="dps", bufs=8, space="PSUM") as dense_ps,
    ):
        wd_sb = wd_pool.tile([P, NHC, DCOL], MDT, name="wd_sb")
        nc.scalar.dma_start(wd_sb[:], wdT[:].rearrange("(c p) o -> p c o", p=P))
        resid_sb = []
        for ct in range(DCOL // P):
            t = resid_pool.tile([P, S], F32, name=f"resid{ct}")
            nc.scalar.dma_start(t[:], residT[ct * P : (ct + 1) * P, :])
            resid_sb.append(t)
        dp = {}
        for ct in range(DCOL // P):
            for s2 in range(NSC):
                dp[(ct, s2)] = dense_ps.tile([P, QCH], F32, name="dp")
        # contraction over the two per-head AG buffers; buffer 0 is ready
        # first, so its 8 feature blocks run while AG(h=1) completes.
        # wdT rows are host-reordered to match (see prepare()).
        for b in range(NH_LOC):
            for c8 in range(N_CORES):
                fc = b * N_CORES + c8
                cf = ctxf_pool.tile([P, S], MDT, name="cf")
                eng = nc.sync if fc % 2 == 0 else nc.scalar
                eng.dma_start(cf[:], ag_out[b][c8 * P : (c8 + 1) * P, :])
                for ct in range(DCOL // P):
                    for s2 in range(NSC):
                        nc.tensor.matmul(
                            dp[(ct, s2)][:],
                            wd_sb[:, fc, ct * P : (ct + 1) * P],
                            cf[:, s2 * QCH : (s2 + 1) * QCH],
                            start=(fc == 0),
                            stop=(fc == NHC - 1),
                        )
        for ct in range(DCOL // P):
            for s2 in range(NSC):
                ot = out_pool.tile([P, QCH], F32, name="ot")
                nc.scalar.activation(
                    ot[:], dp[(ct, s2)][:], AF.Identity, bias=bdense_sb[:, ct : ct + 1]
                )
                nc.vector.tensor_tensor(
                    out=ot[:],
                    in0=ot[:],
                    in1=resid_sb[ct][:, s2 * QCH : (s2 + 1) * QCH],
                    op=ALU.add,
                )
                nc.sync.dma_start(
                    outT[ct * P : (ct + 1) * P, s2 * QCH : (s2 + 1) * QCH], ot[:]
                )


def prepare(hidden_states, residual, alibi, attention_mask, W_qkv, b_qkv, W_dense, b_dense):
    """Host-side input marshalling: slicing per core, zero-FLOP relayouts,
    and mask/alibi analysis for the fixed-shift softmax."""
    inv_norm = 1.0 / math.sqrt(HD)
    hs = np.ascontiguousarray(np.asarray(hidden_states, dtype=np.float32)[0])
    hsT = np.ascontiguousarray(hs.T)
    residT_full = np.ascontiguousarray(np.asarray(residual, dtype=np.float32)[0].T)
    alibi = np.asarray(alibi, dtype=np.float32).reshape(NH, S)
    mask2d = np.asarray(attention_mask).reshape(S, S)  # [q, k], True = masked
    W_qkv = np.asarray(W_qkv, dtype=np.float32)
    b_qkv = np.asarray(b_qkv, dtype=np.float32)
    W_dense = np.asarray(W_dense, dtype=np.float32)
    b_dense = np.asarray(b_dense, dtype=np.float32)

    # block classification on the S.T grid: block (qc, kc) holds
    # k in [kc*128, +128), q in [qc*512, +512)
    block_lists = [[] for _ in range(NQC)]
    mask_tiles = []
    for qc in range(NQC):
        for kc in range(NKC):
            sub = mask2d[qc * QCH : (qc + 1) * QCH, kc * P : (kc + 1) * P]
            if sub.all():
                continue
            if not sub.any():
                block_lists[qc].append((kc, None))
            else:
                mask_tiles.append(
                    np.where(sub.T, np.float32(NEG_BIG), np.float32(0.0))
                )
                block_lists[qc].append((kc, len(mask_tiles) - 1))
    n_part = len(mask_tiles)
    maskadd = (
        np.ascontiguousarray(np.concatenate(mask_tiles, axis=0)) if n_part else None
    )

    # fixed per-q softmax shift: c[h, q] = max over allowed k of alibi[h, k]
    allowed = ~mask2d  # [q, k]
    negc_all = np.zeros((NH, S), dtype=np.float32)
    for h in range(NH):
        masked_vals = np.where(allowed, alibi[h][None, :], -np.inf)
        c = masked_vals.max(axis=1)
        c = np.where(np.isfinite(c), c, 0.0)  # fully-masked rows: degenerate
        negc_all[h] = -c

    in_maps = []
    for core in range(N_CORES):
        heads = [NH_LOC * core + i for i in range(NH_LOC)]
        rows = []
        brows = []
        for h in heads:
            for three in range(3):
                sl = slice(h * 3 * HD + three * HD, h * 3 * HD + (three + 1) * HD)
                w = W_qkv[sl].copy()
                bb_ = b_qkv[sl].copy()
                if three == 0:  # fold 1/sqrt(hd) into the Q projection
                    w *= inv_norm
                    bb_ *= inv_norm
                rows.append(w)
                brows.append(bb_)
        w_sel = np.concatenate(rows, axis=0)  # [768, 2048]
        b_sel = np.concatenate(brows, axis=0)  # [768]
        wqkvT = np.ascontiguousarray(w_sel.T)  # [2048, 768]
        bqkv_c = np.ascontiguousarray(b_sel.reshape(OPC, P).T)  # [128, 6]
        alibi_c = np.ascontiguousarray(
            np.concatenate([alibi[h].reshape(NKC, P).T for h in heads], axis=1)
        )  # [128, 32]: col h_loc*16+kc
        negc_c = np.ascontiguousarray(negc_all[heads])  # [2, 2048]
        # dense weight slice, feature rows reordered to match the two
        # per-head AllGather buffers: (b, c8) -> features [c8*256+b*128, +128)
        wd_raw = W_dense[core * DCOL : (core + 1) * DCOL, :].T  # [2048 feat, 256]
        order = np.concatenate(
            [
                np.arange(c8 * (NH_LOC * HD) + b * HD, c8 * (NH_LOC * HD) + (b + 1) * HD)
                for b in range(NH_LOC)
                for c8 in range(N_CORES)
            ]
        )
        wdT_c = np.ascontiguousarray(wd_raw[order])  # [2048, 256]
        bdense_c = np.ascontiguousarray(
            b_dense[core * DCOL : (core + 1) * DCOL].reshape(DCOL // P, P).T
        )  # [128, 2]
        residT_c = np.ascontiguousarray(
            residT_full[core * DCOL : (core + 1) * DCOL, :]
        )  # [256, 2048]
        m = {
            "hsT": hsT,
            "wqkvT": wqkvT,
            "bqkv": bqkv_c,
            "alibi_b": alibi_c,
            "negc": negc_c,
            "wdT": wdT_c,
            "bdense": bdense_c,
            "residT": residT_c,
            "identity": np.eye(P, dtype=np.float32),
            "ones128": np.ones((P, P), dtype=np.float32),
        }
        if n_part:
            m["maskadd"] = maskadd
        in_maps.append(m)
    return block_lists, n_part, in_maps


def assemble(results):
    shards = [results[c]["outT"] for c in range(N_CORES)]  # [256, 2048] each
    outT = np.concatenate(shards, axis=0)  # [2048 cols, 2048 s]
    return np.ascontiguousarray(outT.T).reshape(B, S, H)


_cache = {}


def kernel(**inputs) -> np.ndarray:
    block_lists, n_part, in_maps = prepare(**inputs)
    key = (tuple(tuple(bl) for bl in block_lists), n_part)
    if key not in _cache:
        _cache[key] = build_program(block_lists, n_part)
    nc = _cache[key]
    res = run_bass_kernel_spmd(nc, in_maps, list(range(N_CORES)), trace=False)
    return assemble(res.results)
